# revision 4
# baseline (speedup 1.0000x reference)
"""Trainium2 Bass kernel for nn_CombinedLoss (MSE + pairwise adaptive-boundary
ranking loss over all pairs i<j of B=8192 elements).

Strategy (v2)
-------------
Sort (pred, target) by target on the host (the loss is permutation
invariant); for sorted i<j:  pair_loss[i,j] = relu(P(e) - (p_j - p_i)),
e = t_j - t_i >= 0, P(e) = BETA*e/(1+GAMMA*e) ~ degree-5 Taylor polynomial.
Expanding P in powers of t_j makes m[i,j] a rank-7 product evaluated by the
TensorEngine (K=21 after bf16 hi/lo split-precision), and one fused
instruction per window (ACT Relu+accum or DVE max0+accum) reduces
sum(relu(m)).

v2 layout:
 * Per-core column ROTATION: core c's V columns are shifted by -128c so each
   slot's needed range starts exactly at its first 1024-col window.  The
   below-diagonal junk that rides along in the first window of each slot is
   the in-block lower triangle; its relu-sum is computed on the host in
   float64 and subtracted (no device-side masks at all).  Columns past the
   logical end hold a sentinel (-1e38 via the p-row) so relu kills them.
 * 4x PE row tiling: K=21 <= 32, so four matmuls run CONCURRENTLY in the
   128x128 array via tile_position=(32g, 0).  Group g owns slots {g, 7-g}
   (9 windows each); V/A are replicated at partition bases 0/32/64/96.
 * Reducers alternate ACT/DVE (18/18), double-buffered PSUM pools
   (4 x [128,1024] f32 tiles = all 8 PSUM banks).
"""

import numpy as np
from math import comb

B = 8192
NCORES = 8
NSLOTS = 8
GROUPS = 4
D = 5           # polynomial degree (truncation err ~ BETA*GAMMA^5 ~ 3e-6)
KDIM = D + 2    # logical contraction rows: ones, t^1..t^5, p
# fp32 matmul is ~5x slower on the PE; use bf16 split-precision instead:
# m = Ahi.Vhi + Ahi.Vlo + Alo.Vhi  (3 stacked sets; the dropped Alo.Vlo term
# is < ~1e-6 because rows with large values split exactly)
KTOT = 3 * KDIM
BETA = 0.3
GAMMA = 0.1
MSE_WEIGHT = 1.0
RANK_WEIGHT = 1.0
NCHUNKS = 36    # per core: 36 windows of 1024 cols (8+7+..+1 per slot pair)
SENTINEL = 1.0e38

_CACHE: dict = {}


def _poly_coeffs():
    # P(a) = sum_{n=1..D} c_n a^n,  c_n = BETA * (-GAMMA)^(n-1)
    return np.array([BETA * (-GAMMA) ** (n - 1) for n in range(1, D + 1)],
                    dtype=np.float64)


def _schedule():
    """Emission order: steps t=0..8, groups g=0..3.  Group g does slot g's
    windows g..7 first, then slot 7-g's windows 7-g..7."""
    order = []
    for t in range(9):
        for g in range(GROUPS):
            if t < 8 - g:
                order.append((g, 0, g + t))          # (group, slot_sel, window)
            else:
                order.append((g, 1, (7 - g) + (t - (8 - g))))
    assert len(order) == NCHUNKS
    return order


def _build_program():
    import concourse.bass as bass
    import concourse.bacc as bacc
    import concourse.tile as tile
    import concourse.mybir as mybir

    f32 = mybir.dt.float32
    f16 = mybir.dt.bfloat16
    Alu = mybir.AluOpType
    Act = mybir.ActivationFunctionType

    nc = bacc.Bacc("TRN2", target_bir_lowering=False, debug=False,
                   num_devices=NCORES)

    V_d = nc.dram_tensor("V4", [128, B], f16, kind="ExternalInput")
    A_d = nc.dram_tensor("A4", [128, 256], f16, kind="ExternalInput")
    T_d = nc.dram_tensor("T64", [128, 64], f32, kind="ExternalInput")
    P_d = nc.dram_tensor("P64", [128, 64], f32, kind="ExternalInput")
    R_d = nc.dram_tensor("RACC", [128, NCHUNKS], f32, kind="ExternalOutput")
    S_d = nc.dram_tensor("MACC", [128, 1], f32, kind="ExternalOutput")

    with tile.TileContext(nc) as tc:
        with (
            tc.tile_pool(name="const", bufs=1) as cp,
            tc.tile_pool(name="scr", bufs=2) as sp,
            tc.tile_pool(name="scrv", bufs=2) as sv,
            tc.tile_pool(name="psa", bufs=2, space="PSUM") as pa,
            tc.tile_pool(name="psv", bufs=2, space="PSUM") as pv,
        ):
            V_sb = cp.tile([128, B], f16)
            A_sb = cp.tile([128, 256], f16)
            T_sb = cp.tile([128, 64], f32)
            P_sb = cp.tile([128, 64], f32)
            acc = cp.tile([128, NCHUNKS], f32)
            macc = cp.tile([128, 1], f32)

            # DMA order: A first (needed by every matmul), then V in
            # full-partition [128, 1024] window pieces, in consumption order
            # (step t consumes windows <= 3+t).  sync/gpsimd queues; the
            # reducer engines (ACT/DVE) are kept free of DMA sequencer work.
            nc.sync.dma_start(A_sb[:], A_d[:])
            eng = [nc.sync, nc.gpsimd]
            for w in range(8):
                c0, c1 = 1024 * w, 1024 * (w + 1)
                eng[w % 2].dma_start(V_sb[:, c0:c1], V_d[:, c0:c1])
            nc.gpsimd.dma_start(T_sb[:], T_d[:])
            nc.gpsimd.dma_start(P_sb[:], P_d[:])

            # Emit matmuls h-interleaved across groups so consecutive PE
            # instructions target different 32-row array groups (they can
            # run concurrently in the 128x128 array).
            sched = _schedule()
            tiles = {}
            for t in range(9):
                step = sched[4 * t:4 * t + 4]
                for chunk4, (g, sel, w) in enumerate(step):
                    chunk = 4 * t + chunk4
                    on_dve = chunk % 2 == 1
                    pool = pv if on_dve else pa
                    tiles[chunk] = pool.tile([128, 1024], f32,
                                             name=f"ps{chunk}",
                                             tag="pv" if on_dve else "pa")
                for h in range(2):
                    for chunk4, (g, sel, w) in enumerate(step):
                        chunk = 4 * t + chunk4
                        p0, p1 = 32 * g, 32 * g + KTOT
                        lhsT = A_sb[p0:p1, 128 * sel:128 * (sel + 1)]
                        c0 = 1024 * w + 512 * h
                        nc.tensor.matmul(
                            tiles[chunk][:, 512 * h:512 * (h + 1)],
                            lhsT,
                            V_sb[p0:p1, c0:c0 + 512],
                            start=True, stop=True,
                            tile_position=(32 * g, 0),
                        )
                for chunk4, (g, sel, w) in enumerate(step):
                    chunk = 4 * t + chunk4
                    ps = tiles.pop(chunk)
                    on_dve = chunk % 2 == 1
                    out_col = acc[:, chunk:chunk + 1]
                    if on_dve:
                        # accum semantics: out = (in0 op0 s1);
                        # accum_out = reduce_op1(out)  (scalar2 unused)
                        z = sv.tile([128, 1024], f32, tag="zv")
                        nc.vector.tensor_scalar(
                            z[:], ps[:], 0.0, None, op0=Alu.max,
                            op1=Alu.add, accum_out=out_col,
                        )
                    else:
                        # in-place relu on PSUM: ScalarE sits closest to PSUM
                        nc.scalar.activation(
                            ps[:], ps[:], Act.Relu, accum_out=out_col,
                        )

            # MSE last: T/P arrive late and this is off the critical path
            d_sb = sp.tile([128, 64], f32, tag="mse")
            nc.vector.tensor_sub(d_sb[:], P_sb[:], T_sb[:])
            mscr = sp.tile([128, 64], f32, tag="mse")
            nc.scalar.activation(mscr[:], d_sb[:], Act.Square,
                                 accum_out=macc[:])

            nc.sync.dma_start(R_d[:], acc[:])
            nc.sync.dma_start(S_d[:], macc[:])

    nc.compile()
    return nc


def _host_inputs(pred: np.ndarray, target: np.ndarray):
    """Sort by target; build rotated V (powers) + per-core lhsT coeffs;
    compute float64 corrections: in-block junk triangles and exact ties."""
    ts32 = np.sort(target, kind="stable")
    order = np.argsort(target, kind="stable")
    ps32 = pred[order]
    ts = ts32.astype(np.float64)
    ps = ps32.astype(np.float64)

    c = _poly_coeffs()
    V = np.empty((KDIM, B), dtype=np.float64)
    V[0] = 1.0
    for k in range(1, D + 1):
        V[k] = ts ** k
    V[KDIM - 1] = ps

    # A_k(t_i) = sum_{n >= max(k,1)} c_n * C(n,k) * (-t_i)^(n-k)
    Ak = np.zeros((D + 1, B), dtype=np.float64)
    for k in range(0, D + 1):
        for n in range(max(k, 1), D + 1):
            Ak[k] += c[n - 1] * comb(n, k) * (-ts) ** (n - k)
    Ak[0] += ps  # fold +p_i into the constant row

    import ml_dtypes

    def split16(x):
        hi = x.astype(ml_dtypes.bfloat16)
        lo = (x - hi.astype(np.float64)).astype(ml_dtypes.bfloat16)
        return hi, lo

    t64 = ts32.reshape(128, 64)
    p64 = ps32.reshape(128, 64)

    in_maps = []
    for core in range(NCORES):
        shift = 128 * core
        Vrot = np.zeros((KDIM, B), dtype=np.float64)
        Vrot[:, :B - shift] = V[:, shift:]
        Vrot[KDIM - 1, B - shift:] = SENTINEL  # p-row sentinel: m -> -1e38
        Vhi, Vlo = split16(Vrot)
        Vf = np.concatenate([Vhi, Vlo, Vhi], axis=0)  # [KTOT, B] bf16

        V4 = np.zeros((128, B), dtype=ml_dtypes.bfloat16)
        A4 = np.zeros((128, 256), dtype=ml_dtypes.bfloat16)
        for g in range(GROUPS):
            V4[32 * g:32 * g + KTOT] = Vf
            for sel, s in enumerate((g, 7 - g)):
                rows = slice(1024 * s + shift, 1024 * s + shift + 128)
                A = np.empty((KDIM, 128), dtype=np.float64)
                A[:D + 1] = Ak[:, rows]
                A[KDIM - 1] = -1.0
                Ahi, Alo = split16(A)
                Af = np.concatenate([Ahi, Ahi, Alo], axis=0)  # [KTOT, 128]
                A4[32 * g:32 * g + KTOT, 128 * sel:128 * (sel + 1)] = Af
        in_maps.append({"V4": V4, "A4": A4, "T64": t64, "P64": p64})

    # --- float64 corrections -------------------------------------------
    # (1) junk: each slot's first window contains the in-block lower
    # triangle (cols x <= row p), i.e. the 64 diagonal 128x128 blocks'
    # lower triangles incl. diagonal.  m[p,x] = Ppoly(t_x - t_p) + p_p - p_x.
    Tb = ts.reshape(64, 128)
    Pb = ps.reshape(64, 128)
    e = Tb[:, None, :] - Tb[:, :, None]          # e[b,p,x] = t_x - t_p
    pp = np.zeros_like(e)
    for n in range(D, 0, -1):
        pp = (pp + c[n - 1]) * e if n > 1 else pp * e + c[0] * e
    m = pp + (Pb[:, :, None] - Pb[:, None, :])   # [64,128,128]
    tril = np.tril(np.ones((128, 128), dtype=bool))
    junk = np.maximum(m, 0.0)[:, tril].sum()

    # (2) ties: reference gives 0 for pairs with t_i == t_j (sign(0)=0); the
    # kernel computes relu(P(0) - (p_j - p_i)) = relu(p_i - p_j) for the
    # sorted pair i<j.  Subtract exactly, in float64.
    ties = 0.0
    uq, inv, cnt = np.unique(ts32, return_inverse=True, return_counts=True)
    for gi in np.nonzero(cnt > 1)[0]:
        idx = np.nonzero(inv == gi)[0]
        pg = ps[idx]
        diff = pg[:, None] - pg[None, :]          # p_u - p_v
        ties += np.maximum(np.triu(diff, 1), 0.0).sum()

    return in_maps, junk + ties


def kernel(pred: np.ndarray, target: np.ndarray):
    from concourse.bass_utils import run_bass_kernel_spmd

    pred = np.ascontiguousarray(np.asarray(pred, dtype=np.float32))
    target = np.ascontiguousarray(np.asarray(target, dtype=np.float32))
    assert pred.shape == (B,) and target.shape == (B,)

    if "nc" not in _CACHE:
        _CACHE["nc"] = _build_program()
    nc = _CACHE["nc"]

    in_maps, corrections = _host_inputs(pred, target)
    res = run_bass_kernel_spmd(nc, in_maps, list(range(NCORES)))
    _CACHE["last_results"] = res

    total = 0.0
    for core in range(NCORES):
        total += res.results[core]["RACC"].astype(np.float64).sum()
    K = B * (B - 1) // 2
    rank = (total - corrections) / K
    mse = res.results[0]["MACC"].astype(np.float64).sum() / B
    combined = MSE_WEIGHT * mse + RANK_WEIGHT * rank
    return (
        np.float32(combined),
        np.float32(mse),
        np.float32(rank),
    )


# revision 9
# speedup vs baseline: 1.0838x; 1.0838x over previous
"""Trainium2 Bass kernel for nn_CombinedLoss (MSE + pairwise adaptive-boundary
ranking loss over all pairs i<j of B=8192 elements).

Strategy (v2)
-------------
Sort (pred, target) by target on the host (the loss is permutation
invariant); for sorted i<j:  pair_loss[i,j] = relu(P(e) - (p_j - p_i)),
e = t_j - t_i >= 0, P(e) = BETA*e/(1+GAMMA*e) ~ degree-5 Taylor polynomial.
Expanding P in powers of t_j makes m[i,j] a rank-7 product evaluated by the
TensorEngine (K=21 after bf16 hi/lo split-precision), and one fused
instruction per window (ACT Relu+accum or DVE max0+accum) reduces
sum(relu(m)).

v2 layout:
 * Per-core column ROTATION: core c's V columns are shifted by -128c so each
   slot's needed range starts exactly at its first 1024-col window.  The
   below-diagonal junk that rides along in the first window of each slot is
   the in-block lower triangle; its relu-sum is computed on the host in
   float64 and subtracted (no device-side masks at all).  Columns past the
   logical end hold a sentinel (-1e38 via the p-row) so relu kills them.
 * 4x PE row tiling: K=21 <= 32, so four matmuls run CONCURRENTLY in the
   128x128 array via tile_position=(32g, 0).  Group g owns slots {g, 7-g}
   (9 windows each); V/A are replicated at partition bases 0/32/64/96.
 * Reducers alternate ACT/DVE (18/18), double-buffered PSUM pools
   (4 x [128,1024] f32 tiles = all 8 PSUM banks).
"""

import numpy as np
from math import comb

B = 8192
NCORES = 8
NSLOTS = 8
GROUPS = 4
D = 5           # polynomial degree (truncation err ~ BETA*GAMMA^5 ~ 3e-6)
KDIM = D + 2    # logical contraction rows: ones, t^1..t^5, p
# fp32 matmul is ~5x slower on the PE; use bf16 split-precision instead:
# m = Ahi.Vhi + Ahi.Vlo + Alo.Vhi  (3 stacked sets; the dropped Alo.Vlo term
# is < ~1e-6 because rows with large values split exactly)
KTOT = 3 * KDIM
BETA = 0.3
GAMMA = 0.1
MSE_WEIGHT = 1.0
RANK_WEIGHT = 1.0
NCHUNKS = 36    # per core: 36 windows of 1024 cols (8+7+..+1 per slot pair)
SENTINEL = 1.0e38

_CACHE: dict = {}


def _poly_coeffs():
    # P(a) = sum_{n=1..D} c_n a^n,  c_n = BETA * (-GAMMA)^(n-1)
    return np.array([BETA * (-GAMMA) ** (n - 1) for n in range(1, D + 1)],
                    dtype=np.float64)


def _schedule():
    """Emission order: steps t=0..8, groups g=0..3.  Group g does slot g's
    windows g..7 first, then slot 7-g's windows 7-g..7."""
    order = []
    for t in range(9):
        for g in range(GROUPS):
            if t < 8 - g:
                order.append((g, 0, g + t))          # (group, slot_sel, window)
            else:
                order.append((g, 1, (7 - g) + (t - (8 - g))))
    assert len(order) == NCHUNKS
    return order


def _build_program():
    import concourse.bass as bass
    import concourse.bacc as bacc
    import concourse.tile as tile
    import concourse.mybir as mybir

    f32 = mybir.dt.float32
    f16 = mybir.dt.bfloat16
    Alu = mybir.AluOpType
    Act = mybir.ActivationFunctionType

    nc = bacc.Bacc("TRN2", target_bir_lowering=False, debug=False,
                   num_devices=NCORES)

    V_d = nc.dram_tensor("V4", [128, B], f16, kind="ExternalInput")
    A_d = nc.dram_tensor("A4", [128, 256], f16, kind="ExternalInput")
    T_d = nc.dram_tensor("T64", [128, 64], f32, kind="ExternalInput")
    P_d = nc.dram_tensor("P64", [128, 64], f32, kind="ExternalInput")
    R_d = nc.dram_tensor("RACC", [128, NCHUNKS + 1], f32,
                         kind="ExternalOutput")

    with tile.TileContext(nc) as tc:
        with (
            tc.tile_pool(name="const", bufs=1) as cp,
            tc.tile_pool(name="scr", bufs=2) as sp,
            tc.tile_pool(name="scrv", bufs=2) as sv,
            tc.tile_pool(name="psa", bufs=2, space="PSUM") as pa,
            tc.tile_pool(name="psv", bufs=2, space="PSUM") as pv,
        ):
            V_sb = cp.tile([128, B], f16)
            A_sb = cp.tile([128, 256], f16)
            T_sb = cp.tile([128, 64], f32)
            P_sb = cp.tile([128, 64], f32)
            acc = cp.tile([128, NCHUNKS + 1], f32)

            # DMA order: A first (needed by every matmul), then per-group
            # 21-row strips: tiny h0 primes on sync (HWDGE), h1 halves +
            # T/P on gpsimd (SWDGE), then the per-group bulk remainder.
            # The reducer engines (ACT/DVE) stay free of input-DMA work.
            nc.sync.dma_start(A_sb[:], A_d[:])
            for g in range(GROUPS):
                p0, p1 = 32 * g, 32 * g + KTOT
                c0 = 1024 * g
                nc.sync.dma_start(V_sb[p0:p1, c0:c0 + 512],
                                  V_d[p0:p1, c0:c0 + 512])
            for g in range(GROUPS):
                p0, p1 = 32 * g, 32 * g + KTOT
                c0 = 1024 * g + 512
                nc.gpsimd.dma_start(V_sb[p0:p1, c0:c0 + 512],
                                    V_d[p0:p1, c0:c0 + 512])
            nc.gpsimd.dma_start(T_sb[:], T_d[:])
            nc.gpsimd.dma_start(P_sb[:], P_d[:])
            eng = [nc.sync, nc.gpsimd]
            for g in range(GROUPS):
                p0, p1 = 32 * g, 32 * g + KTOT
                c0 = 1024 * (g + 1)
                cm = (c0 + B) // 2
                eng[g % 2].dma_start(V_sb[p0:p1, c0:cm], V_d[p0:p1, c0:cm])
                eng[(g + 1) % 2].dma_start(V_sb[p0:p1, cm:B], V_d[p0:p1, cm:B])

            # Emit matmuls h-interleaved across groups so consecutive PE
            # instructions target different 32-row array groups (they can
            # run concurrently in the 128x128 array).
            sched = _schedule()
            tiles = {}
            for t in range(9):
                step = sched[4 * t:4 * t + 4]
                for chunk4, (g, sel, w) in enumerate(step):
                    chunk = 4 * t + chunk4
                    on_dve = chunk % 2 == 1
                    pool = pv if on_dve else pa
                    tiles[chunk] = pool.tile([128, 1024], f32,
                                             name=f"ps{chunk}",
                                             tag="pv" if on_dve else "pa")
                for h in range(2):
                    for chunk4, (g, sel, w) in enumerate(step):
                        chunk = 4 * t + chunk4
                        p0, p1 = 32 * g, 32 * g + KTOT
                        lhsT = A_sb[p0:p1, 128 * sel:128 * (sel + 1)]
                        c0 = 1024 * w + 512 * h
                        nc.tensor.matmul(
                            tiles[chunk][:, 512 * h:512 * (h + 1)],
                            lhsT,
                            V_sb[p0:p1, c0:c0 + 512],
                            start=True, stop=True,
                            tile_position=(32 * g, 0),
                        )
                for chunk4, (g, sel, w) in enumerate(step):
                    chunk = 4 * t + chunk4
                    ps = tiles.pop(chunk)
                    on_dve = chunk % 2 == 1
                    out_col = acc[:, chunk:chunk + 1]
                    if on_dve:
                        # accum semantics: out = (in0 op0 s1);
                        # accum_out = reduce_op1(out)  (scalar2 unused)
                        z = sv.tile([128, 1024], f32, tag="zv")
                        nc.vector.tensor_scalar(
                            z[:], ps[:], 0.0, None, op0=Alu.max,
                            op1=Alu.add, accum_out=out_col,
                        )
                    else:
                        # in-place relu on PSUM: ScalarE sits closest to PSUM
                        nc.scalar.activation(
                            ps[:], ps[:], Act.Relu, accum_out=out_col,
                        )

            # MSE: off the critical path; accum lands in acc col NCHUNKS
            d_sb = sp.tile([128, 64], f32, tag="mse")
            nc.vector.tensor_sub(d_sb[:], P_sb[:], T_sb[:])
            mscr = sp.tile([128, 64], f32, tag="mse")
            nc.scalar.activation(mscr[:], d_sb[:], Act.Square,
                                 accum_out=acc[:, NCHUNKS:NCHUNKS + 1])

            # single output DMA on the scalar engine's (otherwise empty)
            # HWDGE ring so it is not FIFO'd behind input pieces
            nc.scalar.dma_start(R_d[:], acc[:])

    nc.compile()
    return nc


def _host_inputs(pred: np.ndarray, target: np.ndarray):
    """Sort by target; build rotated V (powers) + per-core lhsT coeffs;
    compute float64 corrections: in-block junk triangles and exact ties."""
    ts32 = np.sort(target, kind="stable")
    order = np.argsort(target, kind="stable")
    ps32 = pred[order]
    ts = ts32.astype(np.float64)
    ps = ps32.astype(np.float64)

    c = _poly_coeffs()
    V = np.empty((KDIM, B), dtype=np.float64)
    V[0] = 1.0
    for k in range(1, D + 1):
        V[k] = ts ** k
    V[KDIM - 1] = ps

    # A_k(t_i) = sum_{n >= max(k,1)} c_n * C(n,k) * (-t_i)^(n-k)
    Ak = np.zeros((D + 1, B), dtype=np.float64)
    for k in range(0, D + 1):
        for n in range(max(k, 1), D + 1):
            Ak[k] += c[n - 1] * comb(n, k) * (-ts) ** (n - k)
    Ak[0] += ps  # fold +p_i into the constant row

    import ml_dtypes

    def split16(x):
        hi = x.astype(ml_dtypes.bfloat16)
        lo = (x - hi.astype(np.float64)).astype(ml_dtypes.bfloat16)
        return hi, lo

    t64 = ts32.reshape(128, 64)
    p64 = ps32.reshape(128, 64)

    in_maps = []
    for core in range(NCORES):
        shift = 128 * core
        Vrot = np.zeros((KDIM, B), dtype=np.float64)
        Vrot[:, :B - shift] = V[:, shift:]
        Vrot[KDIM - 1, B - shift:] = SENTINEL  # p-row sentinel: m -> -1e38
        Vhi, Vlo = split16(Vrot)
        Vf = np.concatenate([Vhi, Vlo, Vhi], axis=0)  # [KTOT, B] bf16

        V4 = np.zeros((128, B), dtype=ml_dtypes.bfloat16)
        A4 = np.zeros((128, 256), dtype=ml_dtypes.bfloat16)
        for g in range(GROUPS):
            V4[32 * g:32 * g + KTOT] = Vf
            for sel, s in enumerate((g, 7 - g)):
                rows = slice(1024 * s + shift, 1024 * s + shift + 128)
                A = np.empty((KDIM, 128), dtype=np.float64)
                A[:D + 1] = Ak[:, rows]
                A[KDIM - 1] = -1.0
                Ahi, Alo = split16(A)
                Af = np.concatenate([Ahi, Ahi, Alo], axis=0)  # [KTOT, 128]
                A4[32 * g:32 * g + KTOT, 128 * sel:128 * (sel + 1)] = Af
        in_maps.append({"V4": V4, "A4": A4, "T64": t64, "P64": p64})

    # --- float64 corrections -------------------------------------------
    # (1) junk: each slot's first window contains the in-block lower
    # triangle (cols x <= row p), i.e. the 64 diagonal 128x128 blocks'
    # lower triangles incl. diagonal.  m[p,x] = Ppoly(t_x - t_p) + p_p - p_x.
    Tb = ts.reshape(64, 128)
    Pb = ps.reshape(64, 128)
    e = Tb[:, None, :] - Tb[:, :, None]          # e[b,p,x] = t_x - t_p
    pp = np.zeros_like(e)
    for n in range(D, 0, -1):
        pp = (pp + c[n - 1]) * e if n > 1 else pp * e + c[0] * e
    m = pp + (Pb[:, :, None] - Pb[:, None, :])   # [64,128,128]
    tril = np.tril(np.ones((128, 128), dtype=bool))
    junk = np.maximum(m, 0.0)[:, tril].sum()

    # (2) ties: reference gives 0 for pairs with t_i == t_j (sign(0)=0); the
    # kernel computes relu(P(0) - (p_j - p_i)) = relu(p_i - p_j) for the
    # sorted pair i<j.  Subtract exactly, in float64.
    ties = 0.0
    uq, inv, cnt = np.unique(ts32, return_inverse=True, return_counts=True)
    for gi in np.nonzero(cnt > 1)[0]:
        idx = np.nonzero(inv == gi)[0]
        pg = ps[idx]
        diff = pg[:, None] - pg[None, :]          # p_u - p_v
        ties += np.maximum(np.triu(diff, 1), 0.0).sum()

    return in_maps, junk + ties


def kernel(pred: np.ndarray, target: np.ndarray):
    from concourse.bass_utils import run_bass_kernel_spmd

    pred = np.ascontiguousarray(np.asarray(pred, dtype=np.float32))
    target = np.ascontiguousarray(np.asarray(target, dtype=np.float32))
    assert pred.shape == (B,) and target.shape == (B,)

    if "nc" not in _CACHE:
        _CACHE["nc"] = _build_program()
    nc = _CACHE["nc"]

    in_maps, corrections = _host_inputs(pred, target)
    res = run_bass_kernel_spmd(nc, in_maps, list(range(NCORES)))
    _CACHE["last_results"] = res

    total = 0.0
    for core in range(NCORES):
        total += res.results[core]["RACC"][:, :NCHUNKS].astype(
            np.float64).sum()
    K = B * (B - 1) // 2
    rank = (total - corrections) / K
    mse = res.results[0]["RACC"][:, NCHUNKS].astype(np.float64).sum() / B
    combined = MSE_WEIGHT * mse + RANK_WEIGHT * rank
    return (
        np.float32(combined),
        np.float32(mse),
        np.float32(rank),
    )


# revision 10
# speedup vs baseline: 1.2807x; 1.1817x over previous
"""Trainium2 Bass kernel for nn_CombinedLoss (MSE + pairwise adaptive-boundary
ranking loss over all pairs i<j of B=8192 elements).

Strategy (v2)
-------------
Sort (pred, target) by target on the host (the loss is permutation
invariant); for sorted i<j:  pair_loss[i,j] = relu(P(e) - (p_j - p_i)),
e = t_j - t_i >= 0, P(e) = BETA*e/(1+GAMMA*e) ~ degree-5 Taylor polynomial.
Expanding P in powers of t_j makes m[i,j] a rank-7 product evaluated by the
TensorEngine (K=21 after bf16 hi/lo split-precision), and one fused
instruction per window (ACT Relu+accum or DVE max0+accum) reduces
sum(relu(m)).

v2 layout:
 * Per-core column ROTATION: core c's V columns are shifted by -128c so each
   slot's needed range starts exactly at its first 1024-col window.  The
   below-diagonal junk that rides along in the first window of each slot is
   the in-block lower triangle; its relu-sum is computed on the host in
   float64 and subtracted (no device-side masks at all).  Columns past the
   logical end hold a sentinel (-1e38 via the p-row) so relu kills them.
 * 4x PE row tiling: K=21 <= 32, so four matmuls run CONCURRENTLY in the
   128x128 array via tile_position=(32g, 0).  Group g owns slots {g, 7-g}
   (9 windows each); V/A are replicated at partition bases 0/32/64/96.
 * Reducers alternate ACT/DVE (18/18), double-buffered PSUM pools
   (4 x [128,1024] f32 tiles = all 8 PSUM banks).
"""

import numpy as np
from math import comb

B = 8192
NCORES = 8
NSLOTS = 8
GROUPS = 4
D = 5           # polynomial degree (truncation err ~ BETA*GAMMA^5 ~ 3e-6)
KDIM = D + 2    # logical contraction rows: ones, t^1..t^5, p
# fp32 matmul is ~5x slower on the PE; use bf16 split-precision instead:
# m = Ahi.Vhi + Ahi.Vlo + Alo.Vhi  (3 stacked sets; the dropped Alo.Vlo term
# is < ~1e-6 because rows with large values split exactly)
KTOT = 3 * KDIM
BETA = 0.3
GAMMA = 0.1
MSE_WEIGHT = 1.0
RANK_WEIGHT = 1.0
NCHUNKS = 36    # per core: 36 windows of 1024 cols (8+7+..+1 per slot pair)
SENTINEL = 1.0e38

_CACHE: dict = {}


def _poly_coeffs():
    # P(a) = sum_{n=1..D} c_n a^n,  c_n = BETA * (-GAMMA)^(n-1)
    return np.array([BETA * (-GAMMA) ** (n - 1) for n in range(1, D + 1)],
                    dtype=np.float64)


def _schedule():
    """Emission order: steps t=0..8, groups g=0..3.  Group g does slot g's
    windows g..7 first, then slot 7-g's windows 7-g..7."""
    order = []
    for t in range(9):
        for g in range(GROUPS):
            if t < 8 - g:
                order.append((g, 0, g + t))          # (group, slot_sel, window)
            else:
                order.append((g, 1, (7 - g) + (t - (8 - g))))
    assert len(order) == NCHUNKS
    return order


def _build_program():
    import concourse.bass as bass
    import concourse.bacc as bacc
    import concourse.tile as tile
    import concourse.mybir as mybir

    f32 = mybir.dt.float32
    f16 = mybir.dt.bfloat16
    Alu = mybir.AluOpType
    Act = mybir.ActivationFunctionType

    nc = bacc.Bacc("TRN2", target_bir_lowering=False, debug=False,
                   num_devices=NCORES)

    V_d = nc.dram_tensor("V4", [128, B], f16, kind="ExternalInput")
    A_d = nc.dram_tensor("A4", [128, 256], f16, kind="ExternalInput")
    T_d = nc.dram_tensor("T64", [128, 64], f32, kind="ExternalInput")
    P_d = nc.dram_tensor("P64", [128, 64], f32, kind="ExternalInput")
    R_d = nc.dram_tensor("RACC", [128, NCHUNKS + 1], f32,
                         kind="ExternalOutput")

    with tile.TileContext(nc) as tc:
        with (
            tc.tile_pool(name="const", bufs=1) as cp,
            tc.tile_pool(name="scr", bufs=2) as sp,
            tc.tile_pool(name="scrv", bufs=2) as sv,
            tc.tile_pool(name="psa", bufs=2, space="PSUM") as pa,
            tc.tile_pool(name="psv", bufs=2, space="PSUM") as pv,
        ):
            V_sb = cp.tile([128, B], f16)
            A_sb = cp.tile([128, 256], f16)
            T_sb = cp.tile([128, 64], f32)
            P_sb = cp.tile([128, 64], f32)
            acc = cp.tile([128, NCHUNKS + 1], f32)

            # DMA order: A first (needed by every matmul), then 21-row
            # per-group strips sized/ordered by consumption step: group g
            # consumes windows g..7 at steps 0..7-g.  sync serves groups
            # 0/2, gpsimd groups 1/3 (independent queues); the reducer
            # engines (ACT/DVE) stay free of input-DMA work.
            nc.sync.dma_start(A_sb[:], A_d[:])

            def vdma(e, g, w0, w1):
                p0, p1 = 32 * g, 32 * g + KTOT
                c0, c1 = 1024 * w0, 1024 * w1
                e.dma_start(V_sb[p0:p1, c0:c1], V_d[p0:p1, c0:c1])

            # (group, first window, last window+1) in issue order per queue
            for ge, e in (((0, 2), nc.sync), ((1, 3), nc.gpsimd)):
                ga, gb = ge
                vdma(e, ga, ga, ga + 1)          # prime ga
                vdma(e, gb, gb, gb + 1)          # prime gb
                vdma(e, ga, ga + 1, ga + 3)      # steps 1-2
                vdma(e, gb, gb + 1, min(gb + 3, 8))
                vdma(e, ga, ga + 3, min(ga + 6, 8))  # steps 3-5
                if gb + 3 < 8:
                    vdma(e, gb, gb + 3, 8)
                if ga + 6 < 8:
                    vdma(e, ga, ga + 6, 8)
            nc.gpsimd.dma_start(T_sb[:], T_d[:])
            nc.gpsimd.dma_start(P_sb[:], P_d[:])

            # Emit matmuls h-interleaved across groups so consecutive PE
            # instructions target different 32-row array groups (they can
            # run concurrently in the 128x128 array).
            sched = _schedule()
            tiles = {}
            for t in range(9):
                step = sched[4 * t:4 * t + 4]
                for chunk4, (g, sel, w) in enumerate(step):
                    chunk = 4 * t + chunk4
                    on_dve = chunk % 2 == 1
                    pool = pv if on_dve else pa
                    tiles[chunk] = pool.tile([128, 1024], f32,
                                             name=f"ps{chunk}",
                                             tag="pv" if on_dve else "pa")
                for h in range(2):
                    for chunk4, (g, sel, w) in enumerate(step):
                        chunk = 4 * t + chunk4
                        p0, p1 = 32 * g, 32 * g + KTOT
                        lhsT = A_sb[p0:p1, 128 * sel:128 * (sel + 1)]
                        c0 = 1024 * w + 512 * h
                        nc.tensor.matmul(
                            tiles[chunk][:, 512 * h:512 * (h + 1)],
                            lhsT,
                            V_sb[p0:p1, c0:c0 + 512],
                            start=True, stop=True,
                            tile_position=(32 * g, 0),
                        )
                for chunk4, (g, sel, w) in enumerate(step):
                    chunk = 4 * t + chunk4
                    ps = tiles.pop(chunk)
                    on_dve = chunk % 2 == 1
                    out_col = acc[:, chunk:chunk + 1]
                    if on_dve:
                        # accum semantics: out = (in0 op0 s1);
                        # accum_out = reduce_op1(out)  (scalar2 unused)
                        z = sv.tile([128, 1024], f32, tag="zv")
                        nc.vector.tensor_scalar(
                            z[:], ps[:], 0.0, None, op0=Alu.max,
                            op1=Alu.add, accum_out=out_col,
                        )
                    else:
                        # in-place relu on PSUM: ScalarE sits closest to PSUM
                        nc.scalar.activation(
                            ps[:], ps[:], Act.Relu, accum_out=out_col,
                        )

            # MSE: off the critical path; accum lands in acc col NCHUNKS
            d_sb = sp.tile([128, 64], f32, tag="mse")
            nc.vector.tensor_sub(d_sb[:], P_sb[:], T_sb[:])
            mscr = sp.tile([128, 64], f32, tag="mse")
            nc.scalar.activation(mscr[:], d_sb[:], Act.Square,
                                 accum_out=acc[:, NCHUNKS:NCHUNKS + 1])

            # single output DMA on the scalar engine's (otherwise empty)
            # HWDGE ring so it is not FIFO'd behind input pieces
            nc.scalar.dma_start(R_d[:], acc[:])

    nc.compile()
    return nc


def _host_inputs(pred: np.ndarray, target: np.ndarray):
    """Sort by target; build rotated V (powers) + per-core lhsT coeffs;
    compute float64 corrections: in-block junk triangles and exact ties."""
    ts32 = np.sort(target, kind="stable")
    order = np.argsort(target, kind="stable")
    ps32 = pred[order]
    ts = ts32.astype(np.float64)
    ps = ps32.astype(np.float64)

    c = _poly_coeffs()
    V = np.empty((KDIM, B), dtype=np.float64)
    V[0] = 1.0
    for k in range(1, D + 1):
        V[k] = ts ** k
    V[KDIM - 1] = ps

    # A_k(t_i) = sum_{n >= max(k,1)} c_n * C(n,k) * (-t_i)^(n-k)
    Ak = np.zeros((D + 1, B), dtype=np.float64)
    for k in range(0, D + 1):
        for n in range(max(k, 1), D + 1):
            Ak[k] += c[n - 1] * comb(n, k) * (-ts) ** (n - k)
    Ak[0] += ps  # fold +p_i into the constant row

    import ml_dtypes

    def split16(x):
        hi = x.astype(ml_dtypes.bfloat16)
        lo = (x - hi.astype(np.float64)).astype(ml_dtypes.bfloat16)
        return hi, lo

    t64 = ts32.reshape(128, 64)
    p64 = ps32.reshape(128, 64)

    in_maps = []
    for core in range(NCORES):
        shift = 128 * core
        Vrot = np.zeros((KDIM, B), dtype=np.float64)
        Vrot[:, :B - shift] = V[:, shift:]
        Vrot[KDIM - 1, B - shift:] = SENTINEL  # p-row sentinel: m -> -1e38
        Vhi, Vlo = split16(Vrot)
        Vf = np.concatenate([Vhi, Vlo, Vhi], axis=0)  # [KTOT, B] bf16

        V4 = np.zeros((128, B), dtype=ml_dtypes.bfloat16)
        A4 = np.zeros((128, 256), dtype=ml_dtypes.bfloat16)
        for g in range(GROUPS):
            V4[32 * g:32 * g + KTOT] = Vf
            for sel, s in enumerate((g, 7 - g)):
                rows = slice(1024 * s + shift, 1024 * s + shift + 128)
                A = np.empty((KDIM, 128), dtype=np.float64)
                A[:D + 1] = Ak[:, rows]
                A[KDIM - 1] = -1.0
                Ahi, Alo = split16(A)
                Af = np.concatenate([Ahi, Ahi, Alo], axis=0)  # [KTOT, 128]
                A4[32 * g:32 * g + KTOT, 128 * sel:128 * (sel + 1)] = Af
        in_maps.append({"V4": V4, "A4": A4, "T64": t64, "P64": p64})

    # --- float64 corrections -------------------------------------------
    # (1) junk: each slot's first window contains the in-block lower
    # triangle (cols x <= row p), i.e. the 64 diagonal 128x128 blocks'
    # lower triangles incl. diagonal.  m[p,x] = Ppoly(t_x - t_p) + p_p - p_x.
    Tb = ts.reshape(64, 128)
    Pb = ps.reshape(64, 128)
    e = Tb[:, None, :] - Tb[:, :, None]          # e[b,p,x] = t_x - t_p
    pp = np.zeros_like(e)
    for n in range(D, 0, -1):
        pp = (pp + c[n - 1]) * e if n > 1 else pp * e + c[0] * e
    m = pp + (Pb[:, :, None] - Pb[:, None, :])   # [64,128,128]
    tril = np.tril(np.ones((128, 128), dtype=bool))
    junk = np.maximum(m, 0.0)[:, tril].sum()

    # (2) ties: reference gives 0 for pairs with t_i == t_j (sign(0)=0); the
    # kernel computes relu(P(0) - (p_j - p_i)) = relu(p_i - p_j) for the
    # sorted pair i<j.  Subtract exactly, in float64.
    ties = 0.0
    uq, inv, cnt = np.unique(ts32, return_inverse=True, return_counts=True)
    for gi in np.nonzero(cnt > 1)[0]:
        idx = np.nonzero(inv == gi)[0]
        pg = ps[idx]
        diff = pg[:, None] - pg[None, :]          # p_u - p_v
        ties += np.maximum(np.triu(diff, 1), 0.0).sum()

    return in_maps, junk + ties


def kernel(pred: np.ndarray, target: np.ndarray):
    from concourse.bass_utils import run_bass_kernel_spmd

    pred = np.ascontiguousarray(np.asarray(pred, dtype=np.float32))
    target = np.ascontiguousarray(np.asarray(target, dtype=np.float32))
    assert pred.shape == (B,) and target.shape == (B,)

    if "nc" not in _CACHE:
        _CACHE["nc"] = _build_program()
    nc = _CACHE["nc"]

    in_maps, corrections = _host_inputs(pred, target)
    res = run_bass_kernel_spmd(nc, in_maps, list(range(NCORES)))
    _CACHE["last_results"] = res

    total = 0.0
    for core in range(NCORES):
        total += res.results[core]["RACC"][:, :NCHUNKS].astype(
            np.float64).sum()
    K = B * (B - 1) // 2
    rank = (total - corrections) / K
    mse = res.results[0]["RACC"][:, NCHUNKS].astype(np.float64).sum() / B
    combined = MSE_WEIGHT * mse + RANK_WEIGHT * rank
    return (
        np.float32(combined),
        np.float32(mse),
        np.float32(rank),
    )
